# revision 7
# baseline (speedup 1.0000x reference)
"""KNNGraph (k=16) Bass kernel for 8 NeuronCores.

Input: x (4, 8192, 64) fp32. Output: (src, dst) int32 edge arrays of the
16-NN graph per batch (self included), matching jax.lax.top_k(-d2) order.

Sharding: core c handles batch c//2, query rows (c%2)*4096 ... +4096,
against all 8192 keys of that batch (query-row sharding, keys replicated).

Device (per core, per group of 128 query rows x 4096-key half):
  PE:   8 float32r matmuls (K=66: 64 dims + hi/lo rows folding -|key|^2/2)
        -> PSUM w = q.k - |k|^2/2 (rank-equivalent to -d2/2), 4096 keys
        resident as two 4-bank tiles A (keys 0-2047) and B (keys 2048-4095).
  ACT:  evict last 2560 w values (A[1536:] + B) to SBUF as fp16.
  DVE:  3-level pairwise-max tree (tensor_tensor max) pooling the 4096 w
        down to 512 window maxima (window = {p + 512m}), written as fp16.
  DMA:  ship the pooled array (1024 fp16 per query row) to DRAM.

Host: for each row pick the top-24 pooled windows (a window's max >= the
16th-best w, so the top-16 windows provably contain all true top-16 keys;
24 gives slack for the reduced-precision matmul + fp16 rounding), expand to
24*8 = 192 candidate positions, re-rank them exactly, and emit edges.
A pooled-threshold check detects rows where reduced precision could have
dropped a qualifying window; those rare rows are recomputed exactly.
"""

import numpy as np

N, M, D = 4, 8192, 64
K = 16
NCORES = 8
QROWS = M // 2           # query rows per core
NG = QROWS // 128        # 32 groups of 128 rows
HALF = M // 2            # 4096 keys per PSUM generation
KDIM = 66                # contraction: 64 dims + hi/lo norm rows
POOLW = 8                # pooling window (stride 512 within a half)
NPOOL = HALF // POOLW    # 512 pooled values per half
TOPW = 24                # candidate windows per row taken on host
EPS = 0.12               # w-space safety margin (f32r matmul + fp16 rounding)

_COMPILED = {}


def _build_nc():
    import concourse.bacc as bacc
    import concourse.mybir as mybir
    import concourse.tile as tile

    nc = bacc.Bacc(None)
    f32 = mybir.dt.float32
    f32r = mybir.dt.float32r
    f16 = mybir.dt.float16
    mx = mybir.AluOpType.max

    q_d = nc.declare_dram_parameter("q", [KDIM, QROWS], f32r, isOutput=False)
    kv_d = nc.declare_dram_parameter("kv", [KDIM, M], f32r, isOutput=False)
    pooled_d = nc.declare_dram_parameter(
        "pooled", [NG, 128, 2 * NPOOL], f16, isOutput=True
    )

    with tile.TileContext(nc) as tc:
        with (
            tc.tile_pool(name="singles", bufs=1) as singles,
            tc.tile_pool(name="psum", bufs=1, space="PSUM") as psum,
            tc.tile_pool(name="evict", bufs=2) as evict,
            tc.tile_pool(name="lvls", bufs=2) as lvls,
        ):
            q_sb = singles.tile([KDIM, QROWS], f32r)
            kv0_sb = singles.tile([KDIM, HALF], f32r)
            kv1_sb = singles.tile([KDIM, HALF], f32r)
            nc.sync.dma_start(out=kv0_sb[:], in_=kv_d[:, 0:HALF])
            nc.sync.dma_start(out=q_sb[:], in_=q_d[:])
            nc.sync.dma_start(out=kv1_sb[:], in_=kv_d[:, HALF:M])

            for g in range(NG):
                lhsT = q_sb[:, g * 128:(g + 1) * 128]
                P = lvls.tile([128, 2 * NPOOL], f16, tag="P")
                for h in range(2):
                    kv_h = kv0_sb if h == 0 else kv1_sb
                    # A: keys [0,1024), B: keys [1024,4096) of this half
                    A = psum.tile([128, 1024], f32, tag="A")
                    B = psum.tile([128, 3072], f32, tag="B")
                    for c in range(2):
                        nc.tensor.matmul(
                            A[:, 512 * c:512 * (c + 1)], lhsT,
                            kv_h[:, 512 * c:512 * (c + 1)],
                            start=True, stop=True,
                        )
                    for c in range(6):
                        nc.tensor.matmul(
                            B[:, 512 * c:512 * (c + 1)], lhsT,
                            kv_h[:, 1024 + 512 * c:1024 + 512 * (c + 1)],
                            start=True, stop=True,
                        )
                    # ACT: evict keys [1024,4096) as fp16 (one instruction)
                    E = evict.tile([128, 3072], f16, tag="E")
                    nc.scalar.copy(out=E[:], in_=B[:])
                    # DVE: 3-level pairwise max tree -> 512 pooled fp16
                    # pairs {j, j+2048}: j<1024 PSUM+SBUF, j>=1024 SBUF pairs
                    L1 = lvls.tile([128, 2048], f16, tag="L1")
                    nc.vector.tensor_tensor(
                        out=L1[:, 0:1024], in0=A[:], in1=E[:, 1024:2048], op=mx,
                    )
                    nc.vector.tensor_tensor(
                        out=L1[:, 1024:2048], in0=E[:, 0:1024],
                        in1=E[:, 2048:3072], op=mx,
                    )
                    L2 = lvls.tile([128, 1024], f16, tag="L2")
                    nc.vector.tensor_tensor(
                        out=L2[:], in0=L1[:, 0:1024], in1=L1[:, 1024:2048], op=mx,
                    )
                    nc.vector.tensor_tensor(
                        out=P[:, h * NPOOL:(h + 1) * NPOOL],
                        in0=L2[:, 0:512], in1=L2[:, 512:1024], op=mx,
                    )
                nc.sync.dma_start(out=pooled_d[g], in_=P[:])
    if not nc.is_finalized():
        nc.finalize()
    return nc


def _prep_inputs(x):
    """Per-core input dicts. x: (N, M, D) fp32."""
    x64 = x.astype(np.float64)
    nrm = -0.5 * (x64 * x64).sum(-1)            # (N, M) exact
    hi = nrm.astype(np.float16).astype(np.float32)
    lo = (nrm - hi).astype(np.float32)
    in_maps = []
    for c in range(NCORES):
        b, h2 = c // 2, c % 2
        q = np.zeros((KDIM, QROWS), np.float32)
        q[:D] = x[b, h2 * QROWS:(h2 + 1) * QROWS, :].T
        q[D] = 1.0
        q[D + 1] = 1.0
        kv = np.zeros((KDIM, M), np.float32)
        kv[:D] = x[b].T
        kv[D] = hi[b]
        kv[D + 1] = lo[b]
        in_maps.append({"q": q, "kv": kv})
    return in_maps


def _host_topk_row(x64, b, r):
    """Exact fp64 top-K for one row; returns indices ordered as reference."""
    d2 = ((x64[b] - x64[b, r]) ** 2).sum(-1)
    part = np.argpartition(d2, K)[:K]
    return part[np.argsort(d2[part], kind="stable")]


def kernel(x, k):
    x = np.asarray(x, dtype=np.float32)
    k = int(k)
    assert x.shape == (N, M, D) and k == K

    from concourse.bass_utils import run_bass_kernel_spmd

    if "nc" not in _COMPILED:
        _COMPILED["nc"] = _build_nc()
    nc = _COMPILED["nc"]

    in_maps = _prep_inputs(x)
    res = run_bass_kernel_spmd(nc, in_maps, list(range(NCORES))).results

    # pooled[b, row, win]: win = h*512 + p covers keys h*4096 + p + 512*m
    pooled = np.empty((N, M, 2 * NPOOL), np.float16)
    for c in range(NCORES):
        b, h2 = c // 2, c % 2
        sl = slice(h2 * QROWS, (h2 + 1) * QROWS)
        pooled[b, sl] = res[c]["pooled"].reshape(QROWS, 2 * NPOOL)

    # ---- host: window selection + exact re-rank -----------------------
    import jax
    import jax.numpy as jnp

    cpu = jax.local_devices(backend="cpu")[0]
    x64 = x.astype(np.float64)
    n2_64 = (x64 * x64).sum(-1)                  # (N, M)

    with jax.default_device(cpu):
        topk_fn = _COMPILED.setdefault(
            "topk", jax.jit(lambda p: jax.lax.top_k(p.astype(jnp.float32), TOPW))
        )
        rerank_fn = _COMPILED.setdefault(
            "rerank",
            jax.jit(
                lambda X, n2, cand, q0: n2[cand]
                - 2.0 * jnp.einsum("rcd,rd->rc", X[cand], X[q0])
            ),
        )

        src_parts = []
        for b in range(N):
            pv, wins = topk_fn(pooled[b])        # (M, TOPW)
            wins = np.asarray(wins)
            pv = np.asarray(pv)
            h = wins // NPOOL
            p = wins % NPOOL
            # (M, TOPW, POOLW) candidate positions
            cand = (
                h[:, :, None] * HALF
                + p[:, :, None]
                + np.arange(POOLW, dtype=np.int32)[None, None, :] * NPOOL
            ).reshape(M, TOPW * POOLW).astype(np.int32)
            # exact-ish fp32 scores: d2 - |q|^2 = |k|^2 - 2 q.k  (row-constant
            # offsets don't affect ranking)
            n2_32 = (x[b].astype(np.float32) ** 2).sum(-1)
            sc = np.asarray(
                rerank_fn(x[b], n2_32, cand, np.arange(M, dtype=np.int32))
            )
            # order candidates by ascending index first, then stable-sort by
            # score -> ties broken by lower index, matching jax.lax.top_k
            perm = np.argsort(cand, axis=1, kind="stable")
            cand_s = np.take_along_axis(cand, perm, axis=1)
            sc_s = np.take_along_axis(sc, perm, axis=1)
            order = np.argsort(sc_s, axis=1, kind="stable")[:, : K + 1]
            top_idx = np.take_along_axis(cand_s, order[:, :K], axis=1)
            sc17 = np.take_along_axis(sc_s, order, axis=1)

            # fp64 refinement of rows with near-ties anywhere in the top-17
            shaky = np.nonzero((np.diff(sc17, axis=1) < 1e-3).any(axis=1))[0]
            if shaky.size:
                ks = x64[b][cand_s[shaky]]                     # (s, C, 64)
                sc64 = n2_64[b][cand_s[shaky]] - 2.0 * np.einsum(
                    "rcd,rd->rc", ks, x64[b][shaky]
                )
                o64 = np.argsort(sc64, axis=1, kind="stable")[:, :K]
                top_idx[shaky] = np.take_along_axis(cand_s[shaky], o64, axis=1)
                sc17[shaky, K - 1] = np.take_along_axis(
                    sc64, o64[:, K - 1:K], axis=1
                )[:, 0].astype(np.float32)

            # pooled-threshold widening check: a window whose pooled value
            # beats the 16th-best w minus EPS might hide a true neighbor
            w16 = -0.5 * sc17[:, K - 1].astype(np.float64)
            qual = (pooled[b].astype(np.float32) >= (w16[:, None] - EPS)).sum(1)
            for r in np.nonzero(qual > TOPW)[0]:
                top_idx[r] = _host_topk_row(x64, b, r)

            src_parts.append(top_idx.astype(np.int64) + b * M)

    src = np.concatenate(src_parts).reshape(-1).astype(np.int32)
    dst = np.repeat(np.arange(N * M, dtype=np.int32), K)
    return src, dst


if __name__ == "__main__":
    rng = np.random.default_rng(0)
    xt = rng.standard_normal((N, M, D), dtype=np.float32)
    s, d = kernel(xt, 16)
    print(s[:32], d[:32])


# revision 13
# speedup vs baseline: 1.4644x; 1.4644x over previous
"""KNNGraph (k=16) Bass kernel for 8 NeuronCores.

Input: x (4, 8192, 64) fp32. Output: (src, dst) int32 edge arrays of the
16-NN graph per batch (self included), matching jax.lax.top_k(-d2) order.

Sharding: core c handles batch c//2, query rows (c%2)*4096 ... +4096,
against all 8192 keys of that batch (query-row sharding, keys replicated).

Device (per core, per group of 128 query rows x 4096-key half):
  PE:   8 float32r matmuls (K=66: 64 dims + hi/lo rows folding -|key|^2/2)
        -> PSUM w = q.k - |k|^2/2 (rank-equivalent to -d2/2), 4096 keys
        resident as two 4-bank tiles A (keys 0-2047) and B (keys 2048-4095).
  ACT:  evict last 2560 w values (A[1536:] + B) to SBUF as fp16.
  DVE:  3-level pairwise-max tree (tensor_tensor max) pooling the 4096 w
        down to 512 window maxima (window = {p + 512m}), written as fp16.
  DMA:  ship the pooled array (1024 fp16 per query row) to DRAM.

Host: for each row pick the top-24 pooled windows (a window's max >= the
16th-best w, so the top-16 windows provably contain all true top-16 keys;
24 gives slack for the reduced-precision matmul + fp16 rounding), expand to
24*8 = 192 candidate positions, re-rank them exactly, and emit edges.
A pooled-threshold check detects rows where reduced precision could have
dropped a qualifying window; those rare rows are recomputed exactly.
"""

import numpy as np

N, M, D = 4, 8192, 64
K = 16
NCORES = 8
QROWS = M // 2           # query rows per core
NG = QROWS // 128        # 32 groups of 128 rows
HALF = M // 2            # 4096 keys per PSUM generation
KDIM = 66                # contraction: 64 dims + hi/lo norm rows
POOLW = 8                # pooling window (stride 512 within a half)
NPOOL = HALF // POOLW    # 512 pooled values per half
TOPW = 24                # candidate windows per row taken on host
EPS = 0.12               # w-space safety margin (f32r matmul + fp16 rounding)

_COMPILED = {}


def _build_nc():
    import concourse.bacc as bacc
    import concourse.mybir as mybir
    import concourse.tile as tile

    nc = bacc.Bacc(None)
    f32 = mybir.dt.float32
    f32r = mybir.dt.float32r
    f16 = mybir.dt.float16
    mx = mybir.AluOpType.max

    q_d = nc.declare_dram_parameter("q", [KDIM, QROWS], f16, isOutput=False)
    kv_d = nc.declare_dram_parameter("kv", [KDIM, M], f16, isOutput=False)
    pooled_d = nc.declare_dram_parameter(
        "pooled", [NG, 128, 2 * NPOOL], f16, isOutput=True
    )

    with tile.TileContext(nc) as tc:
        with (
            tc.tile_pool(name="singles", bufs=1) as singles,
            tc.tile_pool(name="psum", bufs=1, space="PSUM") as psum,
            tc.tile_pool(name="evict", bufs=2) as evict,
            tc.tile_pool(name="lvls", bufs=2) as lvls,
        ):
            q_sb = singles.tile([KDIM, QROWS], f16)
            kv0_sb = singles.tile([KDIM, HALF], f16)
            kv1_sb = singles.tile([KDIM, HALF], f16)
            nc.sync.dma_start(out=kv0_sb[:], in_=kv_d[:, 0:HALF])
            nc.sync.dma_start(out=q_sb[:, 0:512], in_=q_d[:, 0:512])
            nc.sync.dma_start(out=kv1_sb[:], in_=kv_d[:, HALF:M])
            nc.sync.dma_start(out=q_sb[:, 512:QROWS], in_=q_d[:, 512:QROWS])

            for g in range(NG):
                lhsT = q_sb[:, g * 128:(g + 1) * 128]
                L1g = lvls.tile([128, 4096], f16, tag="L1g")
                for h in range(2):
                    kv_h = kv0_sb if h == 0 else kv1_sb
                    # keys [0,1536) stay in PSUM; [1536,4096) evicted as fp16
                    A = psum.tile([128, 1536], f32, tag="A")
                    B1 = psum.tile([128, 1024], f32, tag="B1")
                    B2 = psum.tile([128, 1536], f32, tag="B2")
                    for c in range(3):
                        nc.tensor.matmul(
                            A[:, 512 * c:512 * (c + 1)], lhsT,
                            kv_h[:, 512 * c:512 * (c + 1)],
                            start=True, stop=True,
                        )
                    for c in range(2):
                        nc.tensor.matmul(
                            B1[:, 512 * c:512 * (c + 1)], lhsT,
                            kv_h[:, 1536 + 512 * c:1536 + 512 * (c + 1)],
                            start=True, stop=True,
                        )
                    for c in range(3):
                        nc.tensor.matmul(
                            B2[:, 512 * c:512 * (c + 1)], lhsT,
                            kv_h[:, 2560 + 512 * c:2560 + 512 * (c + 1)],
                            start=True, stop=True,
                        )
                    # ACT evictions; E[:, j] = key 1536+j (fp16)
                    E = evict.tile([128, 2560], f16, tag="E")
                    nc.scalar.copy(out=E[:, 0:1024], in_=B1[:])
                    nc.scalar.copy(out=E[:, 1024:2560], in_=B2[:])
                    # DVE level 1: pairs {j, j+2048}
                    nc.vector.tensor_tensor(
                        out=L1g[:, h * 2048:h * 2048 + 1536],
                        in0=A[:], in1=E[:, 512:2048], op=mx,
                    )
                    nc.vector.tensor_tensor(
                        out=L1g[:, h * 2048 + 1536:h * 2048 + 2048],
                        in0=E[:, 0:512], in1=E[:, 2048:2560], op=mx,
                    )
                # DVE levels 2+3, both halves in one instruction via 3-D APs
                L2g = lvls.tile([128, 2048], f16, tag="L2g")
                L1v = L1g[:].rearrange("p (h j) -> p h j", h=2)
                nc.vector.tensor_tensor(
                    out=L2g[:].rearrange("p (h j) -> p h j", h=2),
                    in0=L1v[:, :, 0:1024], in1=L1v[:, :, 1024:2048], op=mx,
                )
                P = lvls.tile([128, 2 * NPOOL], f16, tag="P")
                L2v = L2g[:].rearrange("p (h j) -> p h j", h=2)
                nc.vector.tensor_tensor(
                    out=P[:].rearrange("p (h j) -> p h j", h=2),
                    in0=L2v[:, :, 0:512], in1=L2v[:, :, 512:1024], op=mx,
                )
                nc.sync.dma_start(out=pooled_d[g], in_=P[:])
    if not nc.is_finalized():
        nc.finalize()
    return nc


def _prep_inputs(x):
    """Per-core input dicts. x: (N, M, D) fp32."""
    x64 = x.astype(np.float64)
    nrm = -0.5 * (x64 * x64).sum(-1)            # (N, M) exact
    hi = nrm.astype(np.float16)
    lo = (nrm - hi.astype(np.float64)).astype(np.float16)
    in_maps = []
    for c in range(NCORES):
        b, h2 = c // 2, c % 2
        q = np.zeros((KDIM, QROWS), np.float16)
        q[:D] = x[b, h2 * QROWS:(h2 + 1) * QROWS, :].T
        q[D] = 1.0
        q[D + 1] = 1.0
        kv = np.zeros((KDIM, M), np.float16)
        kv[:D] = x[b].T
        kv[D] = hi[b]
        kv[D + 1] = lo[b]
        in_maps.append({"q": q, "kv": kv})
    return in_maps


def _host_topk_row(x64, b, r):
    """Exact fp64 top-K for one row; returns indices ordered as reference."""
    d2 = ((x64[b] - x64[b, r]) ** 2).sum(-1)
    part = np.argpartition(d2, K)[:K]
    return part[np.argsort(d2[part], kind="stable")]


def kernel(x, k):
    x = np.asarray(x, dtype=np.float32)
    k = int(k)
    assert x.shape == (N, M, D) and k == K

    from concourse.bass_utils import run_bass_kernel_spmd

    if "nc" not in _COMPILED:
        _COMPILED["nc"] = _build_nc()
    nc = _COMPILED["nc"]

    in_maps = _prep_inputs(x)
    res = run_bass_kernel_spmd(nc, in_maps, list(range(NCORES))).results

    # pooled[b, row, win]: win = h*512 + p covers keys h*4096 + p + 512*m
    pooled = np.empty((N, M, 2 * NPOOL), np.float16)
    for c in range(NCORES):
        b, h2 = c // 2, c % 2
        sl = slice(h2 * QROWS, (h2 + 1) * QROWS)
        pooled[b, sl] = res[c]["pooled"].reshape(QROWS, 2 * NPOOL)

    # ---- host: window selection + exact re-rank -----------------------
    import jax
    import jax.numpy as jnp

    cpu = jax.local_devices(backend="cpu")[0]
    x64 = x.astype(np.float64)
    n2_64 = (x64 * x64).sum(-1)                  # (N, M)

    with jax.default_device(cpu):
        topk_fn = _COMPILED.setdefault(
            "topk", jax.jit(lambda p: jax.lax.top_k(p.astype(jnp.float32), TOPW))
        )
        rerank_fn = _COMPILED.setdefault(
            "rerank",
            jax.jit(
                lambda X, n2, cand, q0: n2[cand]
                - 2.0 * jnp.einsum("rcd,rd->rc", X[cand], X[q0])
            ),
        )

        src_parts = []
        for b in range(N):
            pv, wins = topk_fn(pooled[b])        # (M, TOPW)
            wins = np.asarray(wins)
            pv = np.asarray(pv)
            h = wins // NPOOL
            p = wins % NPOOL
            # (M, TOPW, POOLW) candidate positions
            cand = (
                h[:, :, None] * HALF
                + p[:, :, None]
                + np.arange(POOLW, dtype=np.int32)[None, None, :] * NPOOL
            ).reshape(M, TOPW * POOLW).astype(np.int32)
            # exact-ish fp32 scores: d2 - |q|^2 = |k|^2 - 2 q.k  (row-constant
            # offsets don't affect ranking)
            n2_32 = (x[b].astype(np.float32) ** 2).sum(-1)
            sc = np.asarray(
                rerank_fn(x[b], n2_32, cand, np.arange(M, dtype=np.int32))
            )
            # order candidates by ascending index first, then stable-sort by
            # score -> ties broken by lower index, matching jax.lax.top_k
            perm = np.argsort(cand, axis=1, kind="stable")
            cand_s = np.take_along_axis(cand, perm, axis=1)
            sc_s = np.take_along_axis(sc, perm, axis=1)
            order = np.argsort(sc_s, axis=1, kind="stable")[:, : K + 1]
            top_idx = np.take_along_axis(cand_s, order[:, :K], axis=1)
            sc17 = np.take_along_axis(sc_s, order, axis=1)

            # fp64 refinement of rows with near-ties anywhere in the top-17
            shaky = np.nonzero((np.diff(sc17, axis=1) < 1e-3).any(axis=1))[0]
            if shaky.size:
                ks = x64[b][cand_s[shaky]]                     # (s, C, 64)
                sc64 = n2_64[b][cand_s[shaky]] - 2.0 * np.einsum(
                    "rcd,rd->rc", ks, x64[b][shaky]
                )
                o64 = np.argsort(sc64, axis=1, kind="stable")[:, :K]
                top_idx[shaky] = np.take_along_axis(cand_s[shaky], o64, axis=1)
                sc17[shaky, K - 1] = np.take_along_axis(
                    sc64, o64[:, K - 1:K], axis=1
                )[:, 0].astype(np.float32)

            # pooled-threshold widening check: a window whose pooled value
            # beats the 16th-best w minus EPS might hide a true neighbor
            w16 = -0.5 * sc17[:, K - 1].astype(np.float64)
            qual = (pooled[b].astype(np.float32) >= (w16[:, None] - EPS)).sum(1)
            for r in np.nonzero(qual > TOPW)[0]:
                top_idx[r] = _host_topk_row(x64, b, r)

            src_parts.append(top_idx.astype(np.int64) + b * M)

    src = np.concatenate(src_parts).reshape(-1).astype(np.int32)
    dst = np.repeat(np.arange(N * M, dtype=np.int32), K)
    return src, dst


if __name__ == "__main__":
    rng = np.random.default_rng(0)
    xt = rng.standard_normal((N, M, D), dtype=np.float32)
    s, d = kernel(xt, 16)
    print(s[:32], d[:32])
